# revision 35
# baseline (speedup 1.0000x reference)
"""nn_Downsample: depthwise 4x4 stride-2 pad-1 blur downsample on 8 NeuronCores.

Input  x [16, 256, 256, 256] fp32 (NCHW), kernel [4, 4] fp32 (rank-1 FIR).
Output   [16, 256, 128, 128] fp32.

Sharding: pure data parallelism - 2 samples per core across 8 cores.

Per-core program: the conv is separable (kernel = fh x fw outer product),
computed per 256x256 plane as two TensorEngine matmul stages:
  stage 1 (contract h): tmpT[w, i] = sum_h x[h, w] * AT[h, i]
  stage 2 (contract w): out[i, j]  = sum_w tmpT[w, i] * B[w, j]
with AT/B banded matrices holding the taps, all-bf16 (rel err 4e-3 vs the
2e-2 budget).

The whole problem is HBM-bound and, at 8 concurrent cores, limited by the
two-NeuronCores-per-HBM-stack share (~330 GB/s/core effective vs 358 peak;
one core alone runs this same program at ~479us = the cost-model roofline).
Measured on HW (GB/s/core, 8 cores): reads 2KB-desc 339 / 4KB 347 / 8KB
350; writes 512B-desc 274 / 2KB 343; reads+writes are additive (no
overlap). So the layout is chosen entirely to maximize DMA descriptor
contiguity on both ends:

- Input (s1_mode="quad"): partition = (channel-pair, row-quad), so each
  descriptor covers 4 consecutive rows = 4KB. Loads ride SWDGE
  (nc.gpsimd.dma_start), which casts fp32->bf16 during the DMA for free -
  HWDGE cannot cast, and an engine-side cast costs an extra SBUF pass
  (GpSimd tensor_copy measured catastrophically slow at this). Stage 1
  then contracts each channel's 64 quad-partitions in 4 row-phase matmuls
  per w-half (bf16 runs full PE rate at moving dim 128, so the 4-phase
  split costs the same 1024 cycles/channel as the old fp32r concat trick).
- AT's columns are permuted so stage-1 PSUM comes out in (w-half, i%8,
  i//8) order; a strided DVE copy scatters 8 channels into one tmpT tile
  whose free dim is (w-half, i%8, channel, i//8).
- Stage 2 (s2_rows=8): 16 matmuls per 8-channel group (8 i-phases x 2
  w-halves, 128-wide each) produce partition = (channel, i//8), free =
  (i%8, j): each partition holds 8 consecutive output rows of one channel,
  so store descriptors are 4KB (vs 512B for the natural i-partitioned
  layout). Same 256 PE cycles/channel as a 2-matmul stage 2.

DMA routing: input loads on SWDGE, output stores on the scalar (ACT) HWDGE
ring, PSUM->SBUF copies on DVE only. The timing loop uses
For_i(staggered_reset=True) so the back edge has no drain + all-engine
barrier and consecutive iterations overlap like a single-shot pipeline
(-17..23us/iter vs hint_engines alone).
"""

import sys

sys.path.insert(0, "/opt/trn_rl_repo")

import ml_dtypes
import numpy as np

N_CORES = 8

# Final configuration (picked by interleaved A/B timing on hardware;
# see work/ab.py, work/dma_bench.py, work/contention.py)
CFG = dict(
    c_group=8,
    copy_policy="vec",
    in_dma="sync",
    out_dma="scalar",
    s2_mode="qsplit",
    s1_mode="quad",
    s2_rows=8,
    cast_mode="swdge",
    loop_mode="stagger",
    in_bands_per_dma=1,
    split_in=2,
    in_qw_merge=True,
    xbufs=4,
    xbbufs=8,
    obufs=4,
    tbufs=8,
    ps1bufs=4,
    ps2bufs=2,
)

_RUNNER_CACHE = {}


def _factor_kernel(k):
    k = np.asarray(k, dtype=np.float64)
    canon = np.outer([1.0, 3.0, 3.0, 1.0], [1.0, 3.0, 3.0, 1.0]) / 64.0
    if np.allclose(k, canon, rtol=1e-5, atol=1e-8):
        f = np.array([1.0, 3.0, 3.0, 1.0]) / 8.0
        return f, f
    u, s, vt = np.linalg.svd(k)
    fh = u[:, 0] * np.sqrt(s[0])
    fw = vt[0] * np.sqrt(s[0])
    if fh.sum() < 0:
        fh, fw = -fh, -fw
    return fh, fw


def _band_matrices(fh, fw, H=256, W=256):
    HO, WO = H // 2, W // 2
    AT = np.zeros((H, HO), dtype=np.float32)
    for i in range(HO):
        for a in range(4):
            h = 2 * i - 1 + a
            if 0 <= h < H:
                AT[h, i] = fh[a]
    B = np.zeros((W, WO), dtype=np.float32)
    for j in range(WO):
        for b in range(4):
            w = 2 * j - 1 + b
            if 0 <= w < W:
                B[w, j] = fw[b]
    return AT, B


def _weight_inputs(kernel):
    fh, fw = _factor_kernel(kernel)
    AT, B = _band_matrices(fh, fw)
    B0, B1 = B[:128], B[128:]
    ATe, ATo = AT[0::2], AT[1::2]
    # qsplit column order: position k = q*32 + a holds output row i = 4a + q
    perm = np.array([4 * (k % 32) + k // 32 for k in range(128)])
    ATeq, AToq = ATe[:, perm], ATo[:, perm]
    w = {
        "AT01": np.ascontiguousarray(np.concatenate([ATe, ATo], axis=1)),
        "AT10": np.ascontiguousarray(np.concatenate([ATo, ATe], axis=1)),
        "B01": np.ascontiguousarray(np.concatenate([B0, B1], axis=1)),
        "B10": np.ascontiguousarray(np.concatenate([B1, B0], axis=1)),
        "AT01q": np.ascontiguousarray(np.concatenate([ATeq, AToq], axis=1)),
        "AT10q": np.ascontiguousarray(np.concatenate([AToq, ATeq], axis=1)),
        "B0h": np.ascontiguousarray(B0.astype(ml_dtypes.bfloat16)),
        "B1h": np.ascontiguousarray(B1.astype(ml_dtypes.bfloat16)),
    }
    # quad-row stage-1: per q-phase rhs holds AT rows 4p+q, qsplit col order,
    # duplicated into both partition halves (matmul needs lhs/rhs base
    # partitions equal; the lhs channel lives in partitions 0:64 or 64:128)
    # r8 variant: column position k = q*16 + a holds output row i = 8a + q
    perm8 = np.array([8 * (k % 16) + k // 16 for k in range(128)])
    for q in range(4):
        atq = AT[q::4][:, perm].astype(ml_dtypes.bfloat16)
        w[f"ATq{q}"] = np.ascontiguousarray(np.concatenate([atq, atq], axis=0))
        atq8 = AT[q::4][:, perm8].astype(ml_dtypes.bfloat16)
        w[f"ATq{q}r8"] = np.ascontiguousarray(
            np.concatenate([atq8, atq8], axis=0))
    return w


def _wnames(s2_mode, s1_mode="pair", s2_rows=4):
    if s1_mode == "quad":
        sfx = "r8" if s2_rows == 8 else ""
        return [f"ATq0{sfx}", f"ATq1{sfx}", f"ATq2{sfx}", f"ATq3{sfx}",
                "B0h", "B1h"]
    if s2_mode == "qsplit":
        return ["AT01q", "AT10q", "B0h", "B1h"]
    return ["AT01", "AT10", "B01", "B10"]


def _build_nc(
    *,
    loop_iters=None,
    c_group=8,
    copy_policy="vec",
    in_dma="sync",
    out_dma="scalar",
    s2_mode="qsplit",
    s1_mode="pair",
    s2_rows=4,
    cast_mode="pool",
    loop_mode="hints",
    in_bands_per_dma=1,
    split_in=1,
    in_qw_merge=True,
    xbufs=4,
    xbbufs=6,
    obufs=4,
    tbufs=8,
    ps1bufs=4,
    ps2bufs=4,
    n_samples=2,
    C=256,
):
    import concourse.mybir as mybir
    import concourse.tile as tile
    from concourse import bacc

    F32 = mybir.dt.float32
    F32R = mybir.dt.float32r
    BF16 = mybir.dt.bfloat16

    H = W = 256
    HO = WO = 128
    s1_dt = F32R
    s2_dt = BF16 if s2_mode == "qsplit" else F32R
    timing = loop_iters is not None

    nc = bacc.Bacc("TRN2", target_bir_lowering=False)
    if timing:
        x_d = nc.dram_tensor("x", [n_samples, C, H, W], F32, kind="Internal")
        y_d = nc.dram_tensor("y", [n_samples, C, HO, WO], F32, kind="Internal")
        done_d = nc.dram_tensor("done", [1, 1], F32, kind="ExternalOutput")
    else:
        x_d = nc.dram_tensor("x", [n_samples, C, H, W], F32, kind="ExternalInput")
        y_d = nc.dram_tensor("y", [n_samples, C, HO, WO], F32,
                             kind="ExternalOutput")

    wnames = _wnames(s2_mode, s1_mode, s2_rows)
    w_d = {}
    for n in wnames:
        if n.startswith("ATq"):
            w_d[n] = nc.dram_tensor(n, [128, 128], BF16, kind="ExternalInput")
        elif n.startswith("B") and n.endswith("h"):
            w_d[n] = nc.dram_tensor(n, [128, 128], BF16, kind="ExternalInput")
        else:
            w_d[n] = nc.dram_tensor(n, [128, 256], F32, kind="ExternalInput")

    with tile.TileContext(nc) as tc:
        with (
            tc.tile_pool(name="wpool", bufs=1) as wpool,
            tc.tile_pool(name="xpool", bufs=xbufs) as xpool,
            tc.tile_pool(name="xbpool", bufs=xbbufs) as xbpool,
            tc.tile_pool(name="tpool", bufs=tbufs) as tpool,
            tc.tile_pool(name="opool", bufs=obufs) as opool,
            tc.tile_pool(name="ps1", bufs=ps1bufs, space="PSUM") as ps1pool,
            tc.tile_pool(name="ps2", bufs=ps2bufs, space="PSUM") as ps2pool,
        ):
            wt = {}
            for n in wnames:
                if n.startswith("ATq"):
                    t = wpool.tile([128, 128], BF16, tag=f"w_{n}",
                                   name=f"w_{n}")
                    nc.gpsimd.dma_start(out=t[:], in_=w_d[n][:])
                elif n.startswith("B") and n.endswith("h"):
                    t = wpool.tile([128, 128], BF16, tag=f"w_{n}", name=f"w_{n}")
                    nc.gpsimd.dma_start(out=t[:], in_=w_d[n][:])
                else:
                    wdt = s1_dt if n.startswith("AT") else s2_dt
                    t = wpool.tile([128, 256], wdt, tag=f"w_{n}", name=f"w_{n}")
                    nc.gpsimd.dma_start(out=t[:], in_=w_d[n][:].bitcast(wdt))
                wt[n] = t

            eng_i = 0

            def copy_tile(out_ap, in_ap, kind="tmp"):
                nonlocal eng_i
                if copy_policy == "vec":
                    nc.vector.tensor_copy(out=out_ap, in_=in_ap)
                elif copy_policy == "vec_out_scalar":
                    if kind == "out":
                        nc.scalar.copy(out_ap, in_ap)
                    else:
                        nc.vector.tensor_copy(out=out_ap, in_=in_ap)
                elif copy_policy == "alt":
                    if eng_i % 2 == 0:
                        nc.vector.tensor_copy(out=out_ap, in_=in_ap)
                    else:
                        nc.scalar.copy(out_ap, in_ap)
                    eng_i += 1
                else:
                    raise ValueError(copy_policy)

            def in_engine(cg):
                if in_dma == "alt":
                    return nc.sync if cg % 2 == 0 else nc.scalar
                return getattr(nc, in_dma)

            def out_engine(cg):
                if out_dma == "alt":
                    return nc.sync if cg % 2 == 0 else nc.scalar
                return getattr(nc, out_dma)

            def s2_qsplit_store(n, cg, c0, g, tmpT):
                # stage 2 over a 4-channel group in qsplit layout + 2KB store
                po = ps2pool.tile([128, 512], F32, tag="ps2",
                                  name=f"p2_{n}_{cg}_{g}")
                for q in range(4):
                    dst2 = po[:, q * 128 : q * 128 + 128]
                    nc.tensor.matmul(dst2,
                                     tmpT[:, q * 128 : q * 128 + 128],
                                     wt["B0h"][:], start=True, stop=False)
                    nc.tensor.matmul(dst2,
                                     tmpT[:, 512 + q * 128 : 512 + q * 128 + 128],
                                     wt["B1h"][:], start=False, stop=True)
                out_t = opool.tile([128, 512], F32, tag="OUT",
                                   name=f"o_{n}_{cg}_{g}")
                copy_tile(out_t[:], po[:], kind="out")
                dsty = y_d[n, c0 + 4 * g : c0 + 4 * g + 4, :, :].rearrange(
                    "c (a q) j -> (c a) (q j)", q=4)
                out_engine(cg).dma_start(out=dsty, in_=out_t[:])

            def copy_s1_to_tmpT(tmpT, cl4, pst):
                # scatter this channel's (half, q, a) stripes into tmpT's
                # (half, q, c, a) layout, casting fp32 -> bf16
                dstT = tmpT[:].rearrange(
                    "p (h q c a) -> p h q c a",
                    h=2, q=4, c=4, a=32)[:, :, :, cl4, :]
                srcT = pst[:, 0:256].rearrange(
                    "p (h q a) -> p h q a", h=2, q=4, a=32)
                copy_tile(dstT, srcT)

            def body(mark_stages=False):
                n_cgs = C // c_group
                marks = {n_cgs * n_samples // 4, n_cgs * n_samples // 2,
                         3 * n_cgs * n_samples // 4}
                for n in range(n_samples):
                    for cg in range(C // c_group):
                        if mark_stages and (n * n_cgs + cg) in marks:
                            tc.stage_boundary()
                        c0 = cg * c_group

                        if s1_mode == "quad":
                            # 4 rows per partition -> 4KB read descriptors;
                            # bf16 4-phase stage-1 keeps PE parity
                            nb = c_group // 2
                            xtb = xbpool.tile([128, c_group * 1024], BF16,
                                              tag="XB", name=f"xb_{n}_{cg}")
                            if cast_mode == "swdge":
                                bpd = min(in_bands_per_dma, nb)
                                for t0 in range(0, nb, bpd):
                                    # channel 2b+c -> partition (c p),
                                    # free (b, q w); 4KB descriptors either way
                                    src = x_d[n, c0 + 2 * t0 :
                                              c0 + 2 * (t0 + bpd),
                                              :, :].rearrange(
                                        "(b c) (p q) w -> (c p) b (q w)",
                                        c=2, q=4)
                                    dst = xtb[:, t0 * 1024 :
                                              (t0 + bpd) * 1024].rearrange(
                                        "p (b f) -> p b f", b=bpd)
                                    nc.gpsimd.dma_start(out=dst, in_=src)
                            else:
                                xt = xpool.tile([128, c_group * 2 * W], F32,
                                                tag="X", name=f"x_{n}_{cg}")
                                for t in range(nb):
                                    src = x_d[n, c0 + 2 * t : c0 + 2 * t + 2,
                                              :, :].rearrange(
                                        "c (p q) w -> (c p) (q w)", q=4)
                                    in_engine(cg).dma_start(
                                        out=xt[:, t * 1024 : (t + 1) * 1024],
                                        in_=src)
                                for t in range(nb):
                                    s_ = slice(t * 1024, (t + 1) * 1024)
                                    if cast_mode == "pool":
                                        nc.gpsimd.tensor_copy(
                                            out=xtb[:, s_], in_=xt[:, s_])
                                    else:
                                        nc.scalar.copy(xtb[:, s_], xt[:, s_])
                            sfx = "r8" if s2_rows == 8 else ""

                            def s1_quad(c, pst):
                                tb, c2 = c // 2, c % 2
                                for blk in range(2):
                                    dstp = pst[:, blk * 128 : blk * 128 + 128]
                                    for q in range(4):
                                        base = tb * 1024 + q * 256 + blk * 128
                                        nc.tensor.matmul(
                                            dstp,
                                            xtb[c2 * 64 : c2 * 64 + 64,
                                                base : base + 128],
                                            wt[f"ATq{q}{sfx}"][c2 * 64 :
                                                               c2 * 64 + 64, :],
                                            start=(q == 0), stop=(q == 3))

                            if s2_rows == 8:
                                # 8 output rows per partition -> 4KB store
                                # descriptors; one 8-channel group per cg
                                tmpT = tpool.tile([128, 2048], BF16,
                                                  tag="tmpT",
                                                  name=f"t_{n}_{cg}")
                                for c in range(c_group):
                                    pst = ps1pool.tile([128, 256], F32,
                                                       tag="ps1",
                                                       name=f"p1_{n}_{cg}_{c}")
                                    s1_quad(c, pst)
                                    dstT = tmpT[:].rearrange(
                                        "p (h q c a) -> p h q c a",
                                        h=2, q=8, c=8, a=16)[:, :, :, c, :]
                                    srcT = pst[:, 0:256].rearrange(
                                        "p (h q a) -> p h q a",
                                        h=2, q=8, a=16)
                                    copy_tile(dstT, srcT)
                                po = ps2pool.tile([128, 1024], F32, tag="ps2",
                                                  name=f"p2_{n}_{cg}")
                                for q in range(8):
                                    dst2 = po[:, q * 128 : q * 128 + 128]
                                    nc.tensor.matmul(
                                        dst2, tmpT[:, q * 128 : q * 128 + 128],
                                        wt["B0h"][:], start=True, stop=False)
                                    nc.tensor.matmul(
                                        dst2,
                                        tmpT[:, 1024 + q * 128 :
                                             1024 + q * 128 + 128],
                                        wt["B1h"][:], start=False, stop=True)
                                out_t = opool.tile([128, 1024], F32,
                                                   tag="OUT",
                                                   name=f"o_{n}_{cg}")
                                copy_tile(out_t[:], po[:], kind="out")
                                dsty = y_d[n, c0 : c0 + c_group,
                                           :, :].rearrange(
                                    "c (a q) j -> (c a) (q j)", q=8)
                                out_engine(cg).dma_start(out=dsty,
                                                         in_=out_t[:])
                                continue

                            for g in range(c_group // 4):
                                tmpT = tpool.tile([128, 1024], BF16,
                                                  tag="tmpT",
                                                  name=f"t_{n}_{cg}_{g}")
                                for cl4 in range(4):
                                    c = 4 * g + cl4
                                    pst = ps1pool.tile([128, 256], F32,
                                                       tag="ps1",
                                                       name=f"p1_{n}_{cg}_{c}")
                                    s1_quad(c, pst)
                                    copy_s1_to_tmpT(tmpT, cl4, pst)
                                s2_qsplit_store(n, cg, c0, g, tmpT)
                            continue

                        xt = xpool.tile([128, c_group * 2 * W], s1_dt, tag="X",
                                        name=f"x_{n}_{cg}")
                        cpp = c_group // split_in
                        for sp in range(split_in):
                            cl, ch = sp * cpp, (sp + 1) * cpp
                            src = x_d[n, c0 + cl : c0 + ch, :, :]
                            if in_qw_merge:
                                # explicit (q w) merge: the AP normalizer
                                # does not coalesce the two adjacent rows
                                # itself; 2KB descriptors measure ~5% faster
                                src = src.rearrange(
                                    "c (p q) w -> p c (q w)", q=2)
                            else:
                                src = src.rearrange(
                                    "c (p q) w -> p c q w", q=2)
                            in_engine(cg).dma_start(
                                out=xt[:, cl * 2 * W : ch * 2 * W],
                                in_=src.bitcast(s1_dt))

                        if s2_mode == "qsplit":
                            for g in range(c_group // 4):
                                tmpT = tpool.tile([128, 1024], BF16,
                                                  tag="tmpT",
                                                  name=f"t_{n}_{cg}_{g}")
                                for cl4 in range(4):
                                    c = 4 * g + cl4
                                    pst = ps1pool.tile([128, 384], F32,
                                                       tag="ps1",
                                                       name=f"p1_{n}_{cg}_{c}")
                                    for blk in range(2):
                                        base = c * 2 * W + blk * 128
                                        lhsE = xt[:, base : base + 128]
                                        lhsO = xt[:, base + W : base + W + 128]
                                        dst = pst[:, blk * 128 : blk * 128 + 256]
                                        nc.tensor.matmul(dst, lhsE,
                                                         wt["AT01q"][:],
                                                         start=True, stop=False)
                                        nc.tensor.matmul(dst, lhsO,
                                                         wt["AT10q"][:],
                                                         start=False, stop=True)
                                    copy_s1_to_tmpT(tmpT, cl4, pst)
                                s2_qsplit_store(n, cg, c0, g, tmpT)
                            continue

                        out_t = opool.tile([128, c_group * WO], F32, tag="OUT",
                                           name=f"o_{n}_{cg}")

                        if s2_mode == "pair":
                            for cp in range(c_group // 2):
                                po = ps2pool.tile([128, 384], F32, tag="ps2",
                                                  name=f"p2_{n}_{cg}_{cp}")
                                for ci in range(2):
                                    c = 2 * cp + ci
                                    pst = ps1pool.tile([128, 384], F32,
                                                       tag="ps1",
                                                       name=f"p1_{n}_{cg}_{c}")
                                    for blk in range(2):
                                        base = c * 2 * W + blk * 128
                                        lhsE = xt[:, base : base + 128]
                                        lhsO = xt[:, base + W : base + W + 128]
                                        dst = pst[:, blk * 128 : blk * 128 + 256]
                                        nc.tensor.matmul(dst, lhsE,
                                                         wt["AT01"][:],
                                                         start=True, stop=False)
                                        nc.tensor.matmul(dst, lhsO,
                                                         wt["AT10"][:],
                                                         start=False, stop=True)
                                    tmpT = tpool.tile([128, 256], s2_dt,
                                                      tag="tmpT",
                                                      name=f"t_{n}_{cg}_{c}")
                                    copy_tile(tmpT[:], pst[:, 0:256])

                                    dst2 = po[:, ci * 128 : ci * 128 + 256]
                                    nc.tensor.matmul(dst2, tmpT[:, 0:128],
                                                     wt["B01"][:],
                                                     start=True, stop=False)
                                    nc.tensor.matmul(dst2, tmpT[:, 128:256],
                                                     wt["B10"][:],
                                                     start=False, stop=True)
                                c0p = 2 * cp * WO
                                copy_tile(out_t[:, c0p : c0p + 2 * WO],
                                          po[:, 0:256], kind="out")
                        else:
                            for c in range(c_group):
                                pst = ps1pool.tile([128, 512], F32, tag="ps1",
                                                   name=f"p1_{n}_{cg}_{c}")
                                for blk in range(2):
                                    base = c * 2 * W + blk * 128
                                    lhsE = xt[:, base : base + 128]
                                    lhsO = xt[:, base + W : base + W + 128]
                                    dst = pst[:, blk * 256 : blk * 256 + 256]
                                    nc.tensor.matmul(dst, lhsE, wt["AT01"][:],
                                                     start=True, stop=False)
                                    nc.tensor.matmul(dst, lhsO, wt["AT10"][:],
                                                     start=False, stop=True)
                                tmpT = tpool.tile([128, 256], s2_dt, tag="tmpT",
                                                  name=f"t_{n}_{cg}_{c}")
                                copy_tile(tmpT[:, 0:128], pst[:, 0:128])
                                copy_tile(tmpT[:, 128:256], pst[:, 256:384])

                                po = ps2pool.tile([128, 256], F32, tag="ps2",
                                                  name=f"p2_{n}_{cg}_{c}")
                                nc.tensor.matmul(po[:], tmpT[:, 0:128],
                                                 wt["B01"][:],
                                                 start=True, stop=False)
                                nc.tensor.matmul(po[:], tmpT[:, 128:256],
                                                 wt["B10"][:],
                                                 start=False, stop=True)

                                copy_tile(out_t[:, c * WO : c * WO + WO],
                                          po[:, 0:128], kind="out")

                        dsty = y_d[n, c0 : c0 + c_group, :, :].rearrange(
                            "c i j -> i c j")
                        out_engine(cg).dma_start(out=dsty, in_=out_t[:])

            if timing:
                if loop_iters > 1:
                    # The body spans many IRAM blocks per engine; without
                    # branch hints every back-edge pays a ~3-4us I$-miss
                    # stall that the single-shot (graded) kernel never pays.
                    hints = (mybir.EngineType.PE, mybir.EngineType.DVE,
                             mybir.EngineType.SP, mybir.EngineType.Activation,
                             mybir.EngineType.Pool)
                    if loop_mode == "stagger":
                        # staggered sem reset: no drain + all-engine barrier
                        # on the back edge, so consecutive timing iterations
                        # overlap like the single-shot pipeline would
                        with tc.For_i(0, loop_iters, 1, hint_engines=hints,
                                      staggered_reset=True):
                            body(mark_stages=True)
                    else:
                        with tc.For_i(0, loop_iters, 1, hint_engines=hints):
                            body()
                else:
                    body()
                import concourse.mybir as _mybir  # noqa: F401
                dn = wpool.tile([1, 1], F32, name="dn")
                nc.vector.memset(dn[:], 1.0)
                nc.sync.dma_start(out=done_d[:], in_=dn[:])
            else:
                body()

    nc.compile()
    return nc


def _get_nc(**kw):
    key = tuple(sorted(kw.items()))
    if key not in _RUNNER_CACHE:
        _RUNNER_CACHE[key] = _build_nc(**kw)
    return _RUNNER_CACHE[key]


def kernel(x, kernel):
    from concourse.bass_utils import run_bass_kernel_spmd

    x = np.ascontiguousarray(np.asarray(x, dtype=np.float32))
    n_total, C, H, W = x.shape
    assert (n_total, C, H, W) == (16, 256, 256, 256), x.shape
    npc = n_total // N_CORES

    nc = _get_nc(loop_iters=None, n_samples=npc, C=C, **CFG)
    weights = _weight_inputs(np.asarray(kernel, dtype=np.float32))
    weights = {k: weights[k]
               for k in _wnames(CFG["s2_mode"], CFG.get("s1_mode", "pair"),
                                CFG.get("s2_rows", 4))}
    in_maps = [
        {"x": x[i * npc : (i + 1) * npc], **weights} for i in range(N_CORES)
    ]
    last_err = None
    for _attempt in range(3):
        try:
            res = run_bass_kernel_spmd(
                nc, in_maps, core_ids=list(range(N_CORES))
            )
            break
        except Exception as e:  # transient NRT/axon device errors; retry
            last_err = e
    else:
        raise last_err
    return np.concatenate([r["y"] for r in res.results], axis=0)



# revision 45
# speedup vs baseline: 1.0334x; 1.0334x over previous
"""nn_Downsample: depthwise 4x4 stride-2 pad-1 blur downsample on 8 NeuronCores.

Input  x [16, 256, 256, 256] fp32 (NCHW), kernel [4, 4] fp32 (rank-1 FIR).
Output   [16, 256, 128, 128] fp32.

Sharding: pure data parallelism - 2 samples per core across 8 cores.

Per-core program: the conv is separable (kernel = fh x fw outer product),
computed per 256x256 plane as two TensorEngine matmul stages:
  stage 1 (contract h): tmpT[w, i] = sum_h x[h, w] * AT[h, i]
  stage 2 (contract w): out[i, j]  = sum_w tmpT[w, i] * B[w, j]
with AT/B banded matrices holding the taps, all-bf16 (rel err 4e-3 vs the
2e-2 budget).

The whole problem is HBM-bound and, at 8 concurrent cores, limited by the
two-NeuronCores-per-HBM-stack share (~330 GB/s/core effective vs 358 peak;
one core alone runs this same program at ~479us = the cost-model roofline).
Measured on HW (GB/s/core, 8 cores): reads 2KB-desc 339 / 4KB 347 / 8KB
350; writes 512B-desc 274 / 2KB 343; reads+writes are additive (no
overlap). So the layout is chosen entirely to maximize DMA descriptor
contiguity on both ends:

- Input (s1_mode="quad"): partition = (channel-pair, row-quad), so each
  descriptor covers 4 consecutive rows = 4KB. Loads ride SWDGE
  (nc.gpsimd.dma_start), which casts fp32->bf16 during the DMA for free -
  HWDGE cannot cast, and an engine-side cast costs an extra SBUF pass
  (GpSimd tensor_copy measured catastrophically slow at this). Stage 1
  then contracts each channel's 64 quad-partitions in 4 row-phase matmuls
  per w-half (bf16 runs full PE rate at moving dim 128, so the 4-phase
  split costs the same 1024 cycles/channel as the old fp32r concat trick).
- AT's columns are permuted so stage-1 PSUM comes out in (w-half, i%8,
  i//8) order; a strided DVE copy scatters 8 channels into one tmpT tile
  whose free dim is (w-half, i%8, channel, i//8).
- Stage 2 (s2_rows=8): 16 matmuls per 8-channel group (8 i-phases x 2
  w-halves, 128-wide each) produce partition = (channel, i//8), free =
  (i%8, j): each partition holds 8 consecutive output rows of one channel,
  so store descriptors are 4KB (vs 512B for the natural i-partitioned
  layout). Same 256 PE cycles/channel as a 2-matmul stage 2.

DMA routing: input loads on SWDGE, output stores on the scalar (ACT) HWDGE
ring, PSUM->SBUF copies on DVE only. The timing loop uses
For_i(staggered_reset=True) so the back edge has no drain + all-engine
barrier and consecutive iterations overlap like a single-shot pipeline
(-17..23us/iter vs hint_engines alone).
"""

import sys

sys.path.insert(0, "/opt/trn_rl_repo")

import ml_dtypes
import numpy as np

N_CORES = 8

# Final configuration (picked by interleaved A/B timing on hardware;
# see work/ab.py, work/dma_bench.py, work/contention.py)
CFG = dict(
    c_group=8,
    copy_policy="vec",
    in_dma="sync",
    out_dma="scalar",
    s2_mode="qsplit",
    s1_mode="quad",
    s2_rows=8,
    cast_mode="swdge",
    loop_mode="stagger",
    in_bands_per_dma=1,
    split_in=2,
    in_qw_merge=True,
    xbufs=4,
    xbbufs=8,
    obufs=4,
    tbufs=8,
    ps1bufs=4,
    ps2bufs=2,
)

_RUNNER_CACHE = {}


def _factor_kernel(k):
    k = np.asarray(k, dtype=np.float64)
    canon = np.outer([1.0, 3.0, 3.0, 1.0], [1.0, 3.0, 3.0, 1.0]) / 64.0
    if np.allclose(k, canon, rtol=1e-5, atol=1e-8):
        f = np.array([1.0, 3.0, 3.0, 1.0]) / 8.0
        return f, f
    u, s, vt = np.linalg.svd(k)
    fh = u[:, 0] * np.sqrt(s[0])
    fw = vt[0] * np.sqrt(s[0])
    if fh.sum() < 0:
        fh, fw = -fh, -fw
    return fh, fw


def _band_matrices(fh, fw, H=256, W=256):
    HO, WO = H // 2, W // 2
    AT = np.zeros((H, HO), dtype=np.float32)
    for i in range(HO):
        for a in range(4):
            h = 2 * i - 1 + a
            if 0 <= h < H:
                AT[h, i] = fh[a]
    B = np.zeros((W, WO), dtype=np.float32)
    for j in range(WO):
        for b in range(4):
            w = 2 * j - 1 + b
            if 0 <= w < W:
                B[w, j] = fw[b]
    return AT, B


def _weight_inputs(kernel):
    fh, fw = _factor_kernel(kernel)
    AT, B = _band_matrices(fh, fw)
    B0, B1 = B[:128], B[128:]
    ATe, ATo = AT[0::2], AT[1::2]
    # qsplit column order: position k = q*32 + a holds output row i = 4a + q
    perm = np.array([4 * (k % 32) + k // 32 for k in range(128)])
    ATeq, AToq = ATe[:, perm], ATo[:, perm]
    w = {
        "AT01": np.ascontiguousarray(np.concatenate([ATe, ATo], axis=1)),
        "AT10": np.ascontiguousarray(np.concatenate([ATo, ATe], axis=1)),
        "B01": np.ascontiguousarray(np.concatenate([B0, B1], axis=1)),
        "B10": np.ascontiguousarray(np.concatenate([B1, B0], axis=1)),
        "AT01q": np.ascontiguousarray(np.concatenate([ATeq, AToq], axis=1)),
        "AT10q": np.ascontiguousarray(np.concatenate([AToq, ATeq], axis=1)),
        "B0h": np.ascontiguousarray(B0.astype(ml_dtypes.bfloat16)),
        "B1h": np.ascontiguousarray(B1.astype(ml_dtypes.bfloat16)),
    }
    # quad-row stage-1: per q-phase rhs holds AT rows 4p+q, qsplit col order,
    # duplicated into both partition halves (matmul needs lhs/rhs base
    # partitions equal; the lhs channel lives in partitions 0:64 or 64:128)
    # r8 variant: column position k = q*16 + a holds output row i = 8a + q
    perm8 = np.array([8 * (k % 16) + k // 16 for k in range(128)])
    for q in range(4):
        atq = AT[q::4][:, perm].astype(ml_dtypes.bfloat16)
        w[f"ATq{q}"] = np.ascontiguousarray(np.concatenate([atq, atq], axis=0))
        atq8 = AT[q::4][:, perm8].astype(ml_dtypes.bfloat16)
        w[f"ATq{q}r8"] = np.ascontiguousarray(
            np.concatenate([atq8, atq8], axis=0))
        # fp32r quad: [AT_q | AT_{q+1}] wide rhs for the 256-moving overlap
        # trick (second half is garbage, overwritten by the next w-block)
        atqw = np.concatenate(
            [AT[q::4][:, perm8], AT[(q + 1) % 4 :: 4][:, perm8]],
            axis=1).astype(np.float32)
        w[f"ATqw{q}"] = np.ascontiguousarray(
            np.concatenate([atqw, atqw], axis=0))
    # pair-s1 + r8 stage-2: the original even/odd concat weights with the
    # r8 column permutation in both halves
    ATeq8, AToq8 = ATe[:, perm8], ATo[:, perm8]
    w["AT01q8"] = np.ascontiguousarray(np.concatenate([ATeq8, AToq8], axis=1))
    w["AT10q8"] = np.ascontiguousarray(np.concatenate([AToq8, ATeq8], axis=1))
    return w


def _wnames(s2_mode, s1_mode="pair", s2_rows=4, cast_mode="pool"):
    if s1_mode == "quad":
        if cast_mode == "hwdge":
            return ["ATqw0", "ATqw1", "ATqw2", "ATqw3", "B0h", "B1h"]
        sfx = "r8" if s2_rows == 8 else ""
        return [f"ATq0{sfx}", f"ATq1{sfx}", f"ATq2{sfx}", f"ATq3{sfx}",
                "B0h", "B1h"]
    if s2_mode == "qsplit":
        if s2_rows == 8:
            return ["AT01q8", "AT10q8", "B0h", "B1h"]
        return ["AT01q", "AT10q", "B0h", "B1h"]
    return ["AT01", "AT10", "B01", "B10"]


def _build_nc(
    *,
    loop_iters=None,
    c_group=8,
    copy_policy="vec",
    in_dma="sync",
    out_dma="scalar",
    s2_mode="qsplit",
    s1_mode="pair",
    s2_rows=4,
    cast_mode="pool",
    loop_mode="hints",
    in_bands_per_dma=1,
    split_in=1,
    in_qw_merge=True,
    xbufs=4,
    xbbufs=6,
    obufs=4,
    tbufs=8,
    ps1bufs=4,
    ps2bufs=4,
    n_samples=2,
    C=256,
):
    import concourse.mybir as mybir
    import concourse.tile as tile
    from concourse import bacc

    F32 = mybir.dt.float32
    F32R = mybir.dt.float32r
    BF16 = mybir.dt.bfloat16

    H = W = 256
    HO = WO = 128
    s1_dt = F32R
    s2_dt = BF16 if s2_mode == "qsplit" else F32R
    timing = loop_iters is not None

    nc = bacc.Bacc("TRN2", target_bir_lowering=False)
    if timing:
        x_d = nc.dram_tensor("x", [n_samples, C, H, W], F32, kind="Internal")
        y_d = nc.dram_tensor("y", [n_samples, C, HO, WO], F32, kind="Internal")
        done_d = nc.dram_tensor("done", [1, 1], F32, kind="ExternalOutput")
    else:
        x_d = nc.dram_tensor("x", [n_samples, C, H, W], F32, kind="ExternalInput")
        y_d = nc.dram_tensor("y", [n_samples, C, HO, WO], F32,
                             kind="ExternalOutput")

    wnames = _wnames(s2_mode, s1_mode, s2_rows, cast_mode)
    w_d = {}
    for n in wnames:
        if n.startswith("ATq") and not n.startswith("ATqw") \
                and not n.startswith("ATq8"):
            w_d[n] = nc.dram_tensor(n, [128, 128], BF16, kind="ExternalInput")
        elif n.startswith("B") and n.endswith("h"):
            w_d[n] = nc.dram_tensor(n, [128, 128], BF16, kind="ExternalInput")
        else:
            w_d[n] = nc.dram_tensor(n, [128, 256], F32, kind="ExternalInput")

    with tile.TileContext(nc) as tc:
        with (
            tc.tile_pool(name="wpool", bufs=1) as wpool,
            tc.tile_pool(name="xpool", bufs=xbufs) as xpool,
            tc.tile_pool(name="xbpool", bufs=xbbufs) as xbpool,
            tc.tile_pool(name="tpool", bufs=tbufs) as tpool,
            tc.tile_pool(name="opool", bufs=obufs) as opool,
            tc.tile_pool(name="ps1", bufs=ps1bufs, space="PSUM") as ps1pool,
            tc.tile_pool(name="ps2", bufs=ps2bufs, space="PSUM") as ps2pool,
        ):
            wt = {}
            for n in wnames:
                if n.startswith("ATq") and not n.startswith("ATqw"):
                    t = wpool.tile([128, 128], BF16, tag=f"w_{n}",
                                   name=f"w_{n}")
                    nc.gpsimd.dma_start(out=t[:], in_=w_d[n][:])
                elif n.startswith("B") and n.endswith("h"):
                    t = wpool.tile([128, 128], BF16, tag=f"w_{n}", name=f"w_{n}")
                    nc.gpsimd.dma_start(out=t[:], in_=w_d[n][:])
                else:
                    wdt = s1_dt if n.startswith("AT") else s2_dt
                    t = wpool.tile([128, 256], wdt, tag=f"w_{n}", name=f"w_{n}")
                    nc.gpsimd.dma_start(out=t[:], in_=w_d[n][:].bitcast(wdt))
                wt[n] = t

            eng_i = 0

            def copy_tile(out_ap, in_ap, kind="tmp"):
                nonlocal eng_i
                if copy_policy == "vec":
                    nc.vector.tensor_copy(out=out_ap, in_=in_ap)
                elif copy_policy == "vec_out_scalar":
                    if kind == "out":
                        nc.scalar.copy(out_ap, in_ap)
                    else:
                        nc.vector.tensor_copy(out=out_ap, in_=in_ap)
                elif copy_policy == "alt":
                    if eng_i % 2 == 0:
                        nc.vector.tensor_copy(out=out_ap, in_=in_ap)
                    else:
                        nc.scalar.copy(out_ap, in_ap)
                    eng_i += 1
                else:
                    raise ValueError(copy_policy)

            def in_engine(cg):
                if in_dma == "alt":
                    return nc.sync if cg % 2 == 0 else nc.scalar
                return getattr(nc, in_dma)

            def out_engine(cg):
                if out_dma == "alt":
                    return nc.sync if cg % 2 == 0 else nc.scalar
                return getattr(nc, out_dma)

            def s2_qsplit_store(n, cg, c0, g, tmpT):
                # stage 2 over a 4-channel group in qsplit layout + 2KB store
                po = ps2pool.tile([128, 512], F32, tag="ps2",
                                  name=f"p2_{n}_{cg}_{g}")
                for q in range(4):
                    dst2 = po[:, q * 128 : q * 128 + 128]
                    nc.tensor.matmul(dst2,
                                     tmpT[:, q * 128 : q * 128 + 128],
                                     wt["B0h"][:], start=True, stop=False)
                    nc.tensor.matmul(dst2,
                                     tmpT[:, 512 + q * 128 : 512 + q * 128 + 128],
                                     wt["B1h"][:], start=False, stop=True)
                out_t = opool.tile([128, 512], F32, tag="OUT",
                                   name=f"o_{n}_{cg}_{g}")
                copy_tile(out_t[:], po[:], kind="out")
                dsty = y_d[n, c0 + 4 * g : c0 + 4 * g + 4, :, :].rearrange(
                    "c (a q) j -> (c a) (q j)", q=4)
                out_engine(cg).dma_start(out=dsty, in_=out_t[:])

            def copy_s1_to_tmpT(tmpT, cl4, pst):
                # scatter this channel's (half, q, a) stripes into tmpT's
                # (half, q, c, a) layout, casting fp32 -> bf16
                dstT = tmpT[:].rearrange(
                    "p (h q c a) -> p h q c a",
                    h=2, q=4, c=4, a=32)[:, :, :, cl4, :]
                srcT = pst[:, 0:256].rearrange(
                    "p (h q a) -> p h q a", h=2, q=4, a=32)
                copy_tile(dstT, srcT)

            def copy_s1_r8(tmpT, c, pst):
                # r8 variant: tmpT free layout (half, q8, c8, a16)
                dstT = tmpT[:].rearrange(
                    "p (h q c a) -> p h q c a",
                    h=2, q=8, c=8, a=16)[:, :, :, c, :]
                srcT = pst[:, 0:256].rearrange(
                    "p (h q a) -> p h q a", h=2, q=8, a=16)
                copy_tile(dstT, srcT)

            def s2_r8_store(n, cg, c0, tmpT):
                # 8 i-phases x 2 w-halves over an 8-channel group; partition
                # (c, i//8), free (i%8, j) -> 4KB store descriptors
                po = ps2pool.tile([128, 1024], F32, tag="ps2",
                                  name=f"p2_{n}_{cg}")
                for q in range(8):
                    dst2 = po[:, q * 128 : q * 128 + 128]
                    nc.tensor.matmul(
                        dst2, tmpT[:, q * 128 : q * 128 + 128],
                        wt["B0h"][:], start=True, stop=False)
                    nc.tensor.matmul(
                        dst2,
                        tmpT[:, 1024 + q * 128 : 1024 + q * 128 + 128],
                        wt["B1h"][:], start=False, stop=True)
                out_t = opool.tile([128, 1024], F32, tag="OUT",
                                   name=f"o_{n}_{cg}")
                copy_tile(out_t[:], po[:], kind="out")
                dsty = y_d[n, c0 : c0 + c_group, :, :].rearrange(
                    "c (a q) j -> (c a) (q j)", q=8)
                out_engine(cg).dma_start(out=dsty, in_=out_t[:])

            def body(mark_stages=False):
                n_cgs = C // c_group
                marks = {n_cgs * n_samples // 4, n_cgs * n_samples // 2,
                         3 * n_cgs * n_samples // 4}
                for n in range(n_samples):
                    for cg in range(C // c_group):
                        if mark_stages and (n * n_cgs + cg) in marks:
                            tc.stage_boundary()
                        c0 = cg * c_group

                        if s1_mode == "quad" and cast_mode == "hwdge":
                            # 4KB reads on the stable HWDGE ring, no cast:
                            # fp32r 4-phase stage-1 with the 256-moving
                            # overlap trick (2x stage-1 PE, but the 8-core
                            # timeline is DMA-stretched so PE has slack)
                            nb = c_group // 2
                            xt = xpool.tile([128, c_group * 2 * W], s1_dt,
                                            tag="X", name=f"x_{n}_{cg}")
                            for t in range(nb):
                                src = x_d[n, c0 + 2 * t : c0 + 2 * t + 2,
                                          :, :].rearrange(
                                    "c (p q) w -> (c p) (q w)", q=4)
                                in_engine(cg).dma_start(
                                    out=xt[:, t * 1024 : (t + 1) * 1024],
                                    in_=src.bitcast(s1_dt))
                            tmpT = tpool.tile([128, 2048], BF16, tag="tmpT",
                                              name=f"t_{n}_{cg}")
                            for c in range(c_group):
                                tb, c2 = c // 2, c % 2
                                pst = ps1pool.tile([128, 384], F32,
                                                   tag="ps1",
                                                   name=f"p1_{n}_{cg}_{c}")
                                for blk in range(2):
                                    dstp = pst[:, blk * 128 : blk * 128 + 256]
                                    for q in range(4):
                                        base = tb * 1024 + q * 256 + blk * 128
                                        nc.tensor.matmul(
                                            dstp,
                                            xt[c2 * 64 : c2 * 64 + 64,
                                               base : base + 128],
                                            wt[f"ATqw{q}"][c2 * 64 :
                                                           c2 * 64 + 64, :],
                                            start=(q == 0), stop=(q == 3))
                                copy_s1_r8(tmpT, c, pst)
                            s2_r8_store(n, cg, c0, tmpT)
                            continue

                        if s1_mode == "quad":
                            # 4 rows per partition -> 4KB read descriptors;
                            # bf16 4-phase stage-1 keeps PE parity
                            nb = c_group // 2
                            xtb = xbpool.tile([128, c_group * 1024], BF16,
                                              tag="XB", name=f"xb_{n}_{cg}")
                            if cast_mode == "swdge":
                                bpd = min(in_bands_per_dma, nb)
                                for t0 in range(0, nb, bpd):
                                    # channel 2b+c -> partition (c p),
                                    # free (b, q w); 4KB descriptors either way
                                    src = x_d[n, c0 + 2 * t0 :
                                              c0 + 2 * (t0 + bpd),
                                              :, :].rearrange(
                                        "(b c) (p q) w -> (c p) b (q w)",
                                        c=2, q=4)
                                    dst = xtb[:, t0 * 1024 :
                                              (t0 + bpd) * 1024].rearrange(
                                        "p (b f) -> p b f", b=bpd)
                                    nc.gpsimd.dma_start(out=dst, in_=src)
                            else:
                                xt = xpool.tile([128, c_group * 2 * W], F32,
                                                tag="X", name=f"x_{n}_{cg}")
                                for t in range(nb):
                                    src = x_d[n, c0 + 2 * t : c0 + 2 * t + 2,
                                              :, :].rearrange(
                                        "c (p q) w -> (c p) (q w)", q=4)
                                    in_engine(cg).dma_start(
                                        out=xt[:, t * 1024 : (t + 1) * 1024],
                                        in_=src)
                                for t in range(nb):
                                    s_ = slice(t * 1024, (t + 1) * 1024)
                                    if cast_mode == "pool":
                                        nc.gpsimd.tensor_copy(
                                            out=xtb[:, s_], in_=xt[:, s_])
                                    else:
                                        nc.scalar.copy(xtb[:, s_], xt[:, s_])
                            sfx = "r8" if s2_rows == 8 else ""

                            def s1_quad(c, pst):
                                tb, c2 = c // 2, c % 2
                                for blk in range(2):
                                    dstp = pst[:, blk * 128 : blk * 128 + 128]
                                    for q in range(4):
                                        base = tb * 1024 + q * 256 + blk * 128
                                        nc.tensor.matmul(
                                            dstp,
                                            xtb[c2 * 64 : c2 * 64 + 64,
                                                base : base + 128],
                                            wt[f"ATq{q}{sfx}"][c2 * 64 :
                                                               c2 * 64 + 64, :],
                                            start=(q == 0), stop=(q == 3))

                            if s2_rows == 8:
                                # 8 output rows per partition -> 4KB store
                                # descriptors; one 8-channel group per cg
                                tmpT = tpool.tile([128, 2048], BF16,
                                                  tag="tmpT",
                                                  name=f"t_{n}_{cg}")
                                for c in range(c_group):
                                    pst = ps1pool.tile([128, 256], F32,
                                                       tag="ps1",
                                                       name=f"p1_{n}_{cg}_{c}")
                                    s1_quad(c, pst)
                                    copy_s1_r8(tmpT, c, pst)
                                s2_r8_store(n, cg, c0, tmpT)
                                continue

                            for g in range(c_group // 4):
                                tmpT = tpool.tile([128, 1024], BF16,
                                                  tag="tmpT",
                                                  name=f"t_{n}_{cg}_{g}")
                                for cl4 in range(4):
                                    c = 4 * g + cl4
                                    pst = ps1pool.tile([128, 256], F32,
                                                       tag="ps1",
                                                       name=f"p1_{n}_{cg}_{c}")
                                    s1_quad(c, pst)
                                    copy_s1_to_tmpT(tmpT, cl4, pst)
                                s2_qsplit_store(n, cg, c0, g, tmpT)
                            continue

                        xt = xpool.tile([128, c_group * 2 * W], s1_dt, tag="X",
                                        name=f"x_{n}_{cg}")
                        cpp = c_group // split_in
                        for sp in range(split_in):
                            cl, ch = sp * cpp, (sp + 1) * cpp
                            src = x_d[n, c0 + cl : c0 + ch, :, :]
                            if in_qw_merge:
                                # explicit (q w) merge: the AP normalizer
                                # does not coalesce the two adjacent rows
                                # itself; 2KB descriptors measure ~5% faster
                                src = src.rearrange(
                                    "c (p q) w -> p c (q w)", q=2)
                            else:
                                src = src.rearrange(
                                    "c (p q) w -> p c q w", q=2)
                            in_engine(cg).dma_start(
                                out=xt[:, cl * 2 * W : ch * 2 * W],
                                in_=src.bitcast(s1_dt))

                        if s2_mode == "qsplit" and s2_rows == 8:
                            # 2KB HWDGE reads (pair stage-1) + 4KB stores
                            tmpT = tpool.tile([128, 2048], BF16, tag="tmpT",
                                              name=f"t_{n}_{cg}")
                            for c in range(c_group):
                                pst = ps1pool.tile([128, 384], F32,
                                                   tag="ps1",
                                                   name=f"p1_{n}_{cg}_{c}")
                                for blk in range(2):
                                    base = c * 2 * W + blk * 128
                                    lhsE = xt[:, base : base + 128]
                                    lhsO = xt[:, base + W : base + W + 128]
                                    dst = pst[:, blk * 128 : blk * 128 + 256]
                                    nc.tensor.matmul(dst, lhsE,
                                                     wt["AT01q8"][:],
                                                     start=True, stop=False)
                                    nc.tensor.matmul(dst, lhsO,
                                                     wt["AT10q8"][:],
                                                     start=False, stop=True)
                                copy_s1_r8(tmpT, c, pst)
                            s2_r8_store(n, cg, c0, tmpT)
                            continue

                        if s2_mode == "qsplit":
                            for g in range(c_group // 4):
                                tmpT = tpool.tile([128, 1024], BF16,
                                                  tag="tmpT",
                                                  name=f"t_{n}_{cg}_{g}")
                                for cl4 in range(4):
                                    c = 4 * g + cl4
                                    pst = ps1pool.tile([128, 384], F32,
                                                       tag="ps1",
                                                       name=f"p1_{n}_{cg}_{c}")
                                    for blk in range(2):
                                        base = c * 2 * W + blk * 128
                                        lhsE = xt[:, base : base + 128]
                                        lhsO = xt[:, base + W : base + W + 128]
                                        dst = pst[:, blk * 128 : blk * 128 + 256]
                                        nc.tensor.matmul(dst, lhsE,
                                                         wt["AT01q"][:],
                                                         start=True, stop=False)
                                        nc.tensor.matmul(dst, lhsO,
                                                         wt["AT10q"][:],
                                                         start=False, stop=True)
                                    copy_s1_to_tmpT(tmpT, cl4, pst)
                                s2_qsplit_store(n, cg, c0, g, tmpT)
                            continue

                        out_t = opool.tile([128, c_group * WO], F32, tag="OUT",
                                           name=f"o_{n}_{cg}")

                        if s2_mode == "pair":
                            for cp in range(c_group // 2):
                                po = ps2pool.tile([128, 384], F32, tag="ps2",
                                                  name=f"p2_{n}_{cg}_{cp}")
                                for ci in range(2):
                                    c = 2 * cp + ci
                                    pst = ps1pool.tile([128, 384], F32,
                                                       tag="ps1",
                                                       name=f"p1_{n}_{cg}_{c}")
                                    for blk in range(2):
                                        base = c * 2 * W + blk * 128
                                        lhsE = xt[:, base : base + 128]
                                        lhsO = xt[:, base + W : base + W + 128]
                                        dst = pst[:, blk * 128 : blk * 128 + 256]
                                        nc.tensor.matmul(dst, lhsE,
                                                         wt["AT01"][:],
                                                         start=True, stop=False)
                                        nc.tensor.matmul(dst, lhsO,
                                                         wt["AT10"][:],
                                                         start=False, stop=True)
                                    tmpT = tpool.tile([128, 256], s2_dt,
                                                      tag="tmpT",
                                                      name=f"t_{n}_{cg}_{c}")
                                    copy_tile(tmpT[:], pst[:, 0:256])

                                    dst2 = po[:, ci * 128 : ci * 128 + 256]
                                    nc.tensor.matmul(dst2, tmpT[:, 0:128],
                                                     wt["B01"][:],
                                                     start=True, stop=False)
                                    nc.tensor.matmul(dst2, tmpT[:, 128:256],
                                                     wt["B10"][:],
                                                     start=False, stop=True)
                                c0p = 2 * cp * WO
                                copy_tile(out_t[:, c0p : c0p + 2 * WO],
                                          po[:, 0:256], kind="out")
                        else:
                            for c in range(c_group):
                                pst = ps1pool.tile([128, 512], F32, tag="ps1",
                                                   name=f"p1_{n}_{cg}_{c}")
                                for blk in range(2):
                                    base = c * 2 * W + blk * 128
                                    lhsE = xt[:, base : base + 128]
                                    lhsO = xt[:, base + W : base + W + 128]
                                    dst = pst[:, blk * 256 : blk * 256 + 256]
                                    nc.tensor.matmul(dst, lhsE, wt["AT01"][:],
                                                     start=True, stop=False)
                                    nc.tensor.matmul(dst, lhsO, wt["AT10"][:],
                                                     start=False, stop=True)
                                tmpT = tpool.tile([128, 256], s2_dt, tag="tmpT",
                                                  name=f"t_{n}_{cg}_{c}")
                                copy_tile(tmpT[:, 0:128], pst[:, 0:128])
                                copy_tile(tmpT[:, 128:256], pst[:, 256:384])

                                po = ps2pool.tile([128, 256], F32, tag="ps2",
                                                  name=f"p2_{n}_{cg}_{c}")
                                nc.tensor.matmul(po[:], tmpT[:, 0:128],
                                                 wt["B01"][:],
                                                 start=True, stop=False)
                                nc.tensor.matmul(po[:], tmpT[:, 128:256],
                                                 wt["B10"][:],
                                                 start=False, stop=True)

                                copy_tile(out_t[:, c * WO : c * WO + WO],
                                          po[:, 0:128], kind="out")

                        dsty = y_d[n, c0 : c0 + c_group, :, :].rearrange(
                            "c i j -> i c j")
                        out_engine(cg).dma_start(out=dsty, in_=out_t[:])

            if timing:
                if loop_iters > 1:
                    # The body spans many IRAM blocks per engine; without
                    # branch hints every back-edge pays a ~3-4us I$-miss
                    # stall that the single-shot (graded) kernel never pays.
                    hints = (mybir.EngineType.PE, mybir.EngineType.DVE,
                             mybir.EngineType.SP, mybir.EngineType.Activation,
                             mybir.EngineType.Pool)
                    if loop_mode == "stagger":
                        # staggered sem reset: no drain + all-engine barrier
                        # on the back edge, so consecutive timing iterations
                        # overlap like the single-shot pipeline would
                        with tc.For_i(0, loop_iters, 1, hint_engines=hints,
                                      staggered_reset=True):
                            body(mark_stages=True)
                    else:
                        with tc.For_i(0, loop_iters, 1, hint_engines=hints):
                            body()
                else:
                    body()
                import concourse.mybir as _mybir  # noqa: F401
                dn = wpool.tile([1, 1], F32, name="dn")
                nc.vector.memset(dn[:], 1.0)
                nc.sync.dma_start(out=done_d[:], in_=dn[:])
            else:
                body()

    nc.compile()
    return nc


def _get_nc(**kw):
    key = tuple(sorted(kw.items()))
    if key not in _RUNNER_CACHE:
        _RUNNER_CACHE[key] = _build_nc(**kw)
    return _RUNNER_CACHE[key]


def kernel(x, kernel):
    from concourse.bass_utils import run_bass_kernel_spmd

    x = np.ascontiguousarray(np.asarray(x, dtype=np.float32))
    n_total, C, H, W = x.shape
    assert (n_total, C, H, W) == (16, 256, 256, 256), x.shape
    npc = n_total // N_CORES

    nc = _get_nc(loop_iters=None, n_samples=npc, C=C, **CFG)
    weights = _weight_inputs(np.asarray(kernel, dtype=np.float32))
    weights = {k: weights[k]
               for k in _wnames(CFG["s2_mode"], CFG.get("s1_mode", "pair"),
                                CFG.get("s2_rows", 4),
                                CFG.get("cast_mode", "pool"))}
    in_maps = [
        {"x": x[i * npc : (i + 1) * npc], **weights} for i in range(N_CORES)
    ]
    last_err = None
    for _attempt in range(3):
        try:
            res = run_bass_kernel_spmd(
                nc, in_maps, core_ids=list(range(N_CORES))
            )
            break
        except Exception as e:  # transient NRT/axon device errors; retry
            last_err = e
    else:
        raise last_err
    return np.concatenate([r["y"] for r in res.results], axis=0)

